# revision 8
# baseline (speedup 1.0000x reference)
"""Trainium2 Bass kernel for the GSC Vanilla SNN problem.

3-layer LIF spiking net, S=101 timesteps, B=2048 batch, data-parallel over
batch across 8 NeuronCores (256 rows per core).

Math (per layer, per step, spikingjelly LIF with tau=2, v_th=1, hard reset):
    a_t   = v_{t-1} + c_t              (c_t = matmul current incl. bias)
    z_t   = 0.5 * (a_t < 2)            in {0, 0.5}
    v_t   = a_t * z_t                  (= a/2 if no spike else 0)
The true spike s = 1 - 2*z is folded into the next layer's weights
(s @ W + b == z @ (-2W) + (b + colsum(W))); biases ride the matmuls as extra
contraction rows (x gets two ones-rows; layers 2/3 use the always-0.5 pad
lanes h=200/201). Readout accumulates R = sum_t z3_t @ Wr_pad in PSUM
(pair-interleaved [12, 512]); host applies base - (2/S)(R0+R1), log_softmax.

Schedule: layer l is SKEWED by 2*l timesteps, so at any "mega-step" j the
three layers process independent steps (t_l = j - 2l). Their elementwise ops
merge into single wide DVE instructions over layer-major supertiles
([128, 3, 2, 256]), amortizing instruction overhead, and the PE always has a
full pair of matmul work whose inputs were produced a mega-pair earlier.
Timesteps are processed in PAIRS so every matmul runs at N=512 (a full PSUM
bank per m-chunk) and weight loads amortize.

Engines: PE does all matmuls (bf16, f32 PSUM) and, on even steps, also adds
the membrane state into the PSUM current via an identity matmul (so the ACT
evacuation directly yields a = v + c and the DVE add is skipped); ACT
evacuates each pair's PSUM to bf16 SBUF in one [128,1024] activation per
layer; DVE does the remaining adds (TT 2x bf16), the z compares (TS 4x), and
the reset multiplies (TT 2x).
"""

import numpy as np
import ml_dtypes

S = 101
D = 120            # C*M input features
DA = 122           # augmented with 2 ones-rows for hi/lo bias
H = 200
HP = 256           # padded hidden
DOUT = 12
NCORES = 8
B_FULL = 2048
BC = B_FULL // NCORES   # 256 batch rows per core
TB = 8                  # x DMA block (timesteps per DMA)
NL = 3                  # layers
SKEW = 2                # timestep skew between adjacent layers

CFG = {"pe_vadd": True}

_bf16 = ml_dtypes.bfloat16

_BUILD_CACHE = {}


def _build(s_steps, bc, tb, cfg=None):
    """Build + compile the Bass program for one core. Returns nc."""
    import concourse.bacc as bacc
    import concourse.mybir as mybir
    import concourse.tile as tile

    cfg = cfg or CFG
    pe_vadd = cfg.get("pe_vadd", True)
    dt = mybir.dt
    alu = mybir.AluOpType
    P = 128
    B2 = 2 * bc
    n_rpairs = (s_steps + 1) // 2          # real pairs per layer (last may be single)
    n_mpairs = n_rpairs + NL - 1           # mega-pairs

    def pair_len(q):  # steps in real pair q
        return min(2, s_steps - 2 * q)

    nc = bacc.Bacc("TRN2", target_bir_lowering=False, debug=False)

    x_d = nc.dram_tensor("x", [DA, s_steps * bc], dt.bfloat16, kind="ExternalInput")
    w1_d = nc.dram_tensor("w1", [DA, HP], dt.bfloat16, kind="ExternalInput")
    w2_d = nc.dram_tensor("w2", [2, P, HP], dt.bfloat16, kind="ExternalInput")
    w3_d = nc.dram_tensor("w3", [2, P, HP], dt.bfloat16, kind="ExternalInput")
    wr_d = nc.dram_tensor("wr", [2, P, DOUT], dt.bfloat16, kind="ExternalInput")
    eye_d = nc.dram_tensor("eye", [P, P], dt.bfloat16, kind="ExternalInput")
    out_d = nc.dram_tensor("out", [DOUT, B2], dt.float32, kind="ExternalOutput")

    with tile.TileContext(nc) as tc:
        with (
            tc.tile_pool(name="const", bufs=1) as constp,
            tc.tile_pool(name="xp", bufs=3) as xp,
            tc.tile_pool(name="state", bufs=1) as statep,
            tc.tile_pool(name="chp_", bufs=3) as chp_,
            tc.tile_pool(name="zp_", bufs=3) as zp_,
            tc.tile_pool(name="ap_", bufs=3) as ap_,
            tc.tile_pool(name="outp", bufs=1) as outp,
            tc.tile_pool(name="ps", bufs=1, space="PSUM") as psp,
            tc.tile_pool(name="psr", bufs=1, space="PSUM") as psrp,
        ):
            w1 = constp.tile([DA, HP], dt.bfloat16)
            nc.sync.dma_start(w1[:], w1_d[:])
            w2a = constp.tile([P, HP], dt.bfloat16)
            w2b = constp.tile([P, HP], dt.bfloat16)
            nc.sync.dma_start(w2a[:], w2_d[0])
            nc.sync.dma_start(w2b[:], w2_d[1])
            w3a = constp.tile([P, HP], dt.bfloat16)
            w3b = constp.tile([P, HP], dt.bfloat16)
            nc.sync.dma_start(w3a[:], w3_d[0])
            nc.sync.dma_start(w3b[:], w3_d[1])
            wra = constp.tile([P, DOUT], dt.bfloat16)
            wrb = constp.tile([P, DOUT], dt.bfloat16)
            nc.sync.dma_start(wra[:], wr_d[0])
            nc.sync.dma_start(wrb[:], wr_d[1])
            eye = constp.tile([P, P], dt.bfloat16)
            nc.sync.dma_start(eye[:], eye_d[:])

            # persistent membrane state, layer-major: [128, l, m, b]
            st = statep.tile([P, NL, 2, bc], dt.bfloat16)
            nc.vector.memset(st[:], 0.0)

            R = psrp.tile([DOUT, B2], dt.float32)

            w23 = [(w2a, w2b), (w3a, w3b)]
            # per-layer PSUM current pair tiles, created per mega-pair
            xb = None
            zprev_tiles = [None, None]  # z supertiles of mega-pairs J-1, J
            ch_cur = None

            for J in range(n_mpairs):
                # ---- matmul phase: layer l works real pair q = J - l ----
                lcs = [l for l in range(NL) if 0 <= J - l < n_rpairs]
                ch_prev = ch_cur
                ch_cur = chp_.tile([P, NL, 2, 2, bc], dt.bfloat16, name="ch")
                z_cur = zp_.tile([P, NL, 2, 2, bc], dt.bfloat16, name="z")
                c_tiles = {}
                for l in lcs:
                    q = J - l
                    np_ = pair_len(q)
                    W = np_ * bc
                    c = psp.tile([P, 2, B2], dt.float32, name=f"c{l + 1}")
                    c_tiles[l] = (c, q, np_, W)
                    if l == 0:
                        if (2 * q) % tb == 0:
                            t0 = 2 * q
                            ncols = min(tb, s_steps - t0) * bc
                            xb = xp.tile([DA, tb * bc], dt.bfloat16, name="xb")
                            nc.sync.dma_start(
                                xb[:, 0:ncols], x_d[:, t0 * bc : t0 * bc + ncols]
                            )
                        xpair = xb[:, (2 * q % tb) * bc : (2 * q % tb) * bc + W]
                        for m in range(2):
                            nc.tensor.matmul(
                                c[:, m, 0:W], w1[:, m * P : (m + 1) * P], xpair,
                                start=True, stop=not pe_vadd,
                                skip_group_check=True,
                            )
                    else:
                        zsrc = zprev_tiles[-1]  # written during mega-pair J-1
                        wka, wkb = w23[l - 1]
                        for m in range(2):
                            nc.tensor.matmul(
                                c[:, m, 0:W],
                                wka[:, m * P : (m + 1) * P],
                                zsrc[:, l - 1, 0, 0:np_, :],
                                start=True, stop=False,
                                skip_group_check=True,
                            )
                            nc.tensor.matmul(
                                c[:, m, 0:W],
                                wkb[:, m * P : (m + 1) * P],
                                zsrc[:, l - 1, 1, 0:np_, :],
                                start=False, stop=not pe_vadd,
                                skip_group_check=True,
                            )
                    if pe_vadd:
                        # add membrane state into the even-step half of the bank
                        for m in range(2):
                            nc.tensor.matmul(
                                c[:, m, 0:bc], eye[:], st[:, l, m, :],
                                start=False, stop=True,
                                skip_group_check=True,
                            )
                    # evacuate the pair's current to bf16 SBUF in one ACT op
                    nc.scalar.copy(
                        ch_cur[:, l, :, 0:np_, :],
                        c[:, :, 0:W].rearrange("p m (s b) -> p m s b", s=np_),
                    )

                # ---- readout: z3 of real pair q3 = J - NL + 1 ... wait, l=2 ----
                if zprev_tiles[-1] is not None:
                    q3 = (J - 1) - 2
                    if 0 <= q3 < n_rpairs:
                        np3 = pair_len(q3)
                        W3_ = np3 * bc
                        zsrc = zprev_tiles[-1]
                        nc.tensor.matmul(
                            R[:, 0:W3_], wra[:], zsrc[:, 2, 0, 0:np3, :],
                            start=(q3 == 0), stop=False, skip_group_check=True,
                        )
                        nc.tensor.matmul(
                            R[:, 0:W3_], wrb[:], zsrc[:, 2, 1, 0:np3, :],
                            start=False, stop=(q3 == n_rpairs - 1),
                            skip_group_check=True,
                        )

                # ---- elementwise phase: mega-steps 2J, 2J+1 ----
                for p in range(2):
                    j = 2 * J + p
                    lr = [l for l in range(NL)
                          if 0 <= j - SKEW * l < s_steps and (J - l) >= 0
                          and (J - l) < n_rpairs and p < pair_len(J - l)]
                    if not lr:
                        continue
                    l0, l1 = lr[0], lr[-1] + 1
                    L = l1 - l0
                    ch_sl = ch_cur[:, l0:l1, :, p, :]       # [128, L, 2, 256]
                    z_sl = z_cur[:, l0:l1, :, p, :]
                    st_sl = st[:, l0:l1, :, :]
                    if p == 0 and pe_vadd:
                        a_sl = ch_sl                         # a already = v + c
                    else:
                        a = ap_.tile([P, NL, 2, bc], dt.bfloat16, name="a")
                        a_sl = a[:, l0:l1, :, :]
                        nc.vector.tensor_tensor(a_sl, st_sl, ch_sl, op=alu.add)
                    nc.vector.tensor_scalar(
                        z_sl, a_sl, 2.0, 0.5, alu.is_lt, alu.mult
                    )
                    nc.vector.tensor_tensor(st_sl, a_sl, z_sl, op=alu.mult)

                zprev_tiles = [zprev_tiles[-1], z_cur]

            # ---- final readout for the last NL-1 mega-pairs' z3 ----
            for J in (n_mpairs, n_mpairs + 1):
                q3 = (J - 1) - 2
                if 0 <= q3 < n_rpairs:
                    np3 = pair_len(q3)
                    W3_ = np3 * bc
                    zsrc = zprev_tiles[-1] if J == n_mpairs else None
                    if zsrc is None:
                        break
                    nc.tensor.matmul(
                        R[:, 0:W3_], wra[:], zsrc[:, 2, 0, 0:np3, :],
                        start=(q3 == 0), stop=False, skip_group_check=True,
                    )
                    nc.tensor.matmul(
                        R[:, 0:W3_], wrb[:], zsrc[:, 2, 1, 0:np3, :],
                        start=False, stop=(q3 == n_rpairs - 1),
                        skip_group_check=True,
                    )

            out_sb = outp.tile([DOUT, B2], dt.float32)
            nc.vector.tensor_copy(out_sb[:], R[:])
            nc.sync.dma_start(out_d[:], out_sb[:])

    nc.compile()
    return nc


def _get_nc(s_steps=S, bc=BC, tb=TB):
    key = (s_steps, bc, tb)
    if key not in _BUILD_CACHE:
        _BUILD_CACHE[key] = _build(s_steps, bc, tb)
    return _BUILD_CACHE[key]


def _hi_lo(v):
    hi = v.astype(_bf16)
    lo = (v - hi.astype(np.float64)).astype(_bf16)
    return hi, lo


def _prep_weights(W1, b1, W2, b2, W3, b3, Wr, br):
    """Host-side weight packing. Returns (device array dict, host affine base)."""
    P = 128

    def pad(w, rows, cols, scale=1.0):
        w = np.asarray(w, np.float64) * scale
        out = np.zeros((rows, cols), np.float64)
        out[: w.shape[0], : w.shape[1]] = w
        return out

    w1p = pad(W1, DA, HP)
    bh = np.zeros(HP, np.float64)
    bh[:H] = np.asarray(b1, np.float64)
    w1p_bf = w1p.astype(_bf16)
    w1p_bf[D], w1p_bf[D + 1] = _hi_lo(bh)

    def mid(W, b):
        wp = pad(W, HP, HP, scale=-2.0).astype(_bf16)
        bh = np.zeros(HP, np.float64)
        bh[:H] = np.asarray(b, np.float64) + np.asarray(W, np.float64).sum(axis=0)
        hi, lo = _hi_lo(bh)
        wp[H] = 2.0 * hi.astype(np.float64)
        wp[H + 1] = 2.0 * lo.astype(np.float64)
        return wp.reshape(2, P, HP)

    w2p = mid(W2, b2)
    w3p = mid(W3, b3)
    wrp = pad(Wr, HP, DOUT).astype(_bf16).reshape(2, P, DOUT)

    base = (np.asarray(br, np.float64) + np.asarray(Wr, np.float64).sum(axis=0)).astype(
        np.float32
    )
    eye = np.eye(P, dtype=_bf16)
    return {"w1": w1p_bf, "w2": w2p, "w3": w3p, "wr": wrp, "eye": eye}, base


def _prep_x(x):
    """[B,C,S,M] f32 -> per-core [DA, S*bc] bf16 list (with two ones-rows)."""
    x = np.asarray(x, np.float32)
    B = x.shape[0]
    bc = B // NCORES
    xt = np.ascontiguousarray(x.transpose(1, 3, 2, 0)).reshape(D, S, B).astype(_bf16)
    outs = []
    for i in range(NCORES):
        xc = np.ones((DA, S * bc), dtype=_bf16)
        xc[:D] = xt[:, :, i * bc : (i + 1) * bc].reshape(D, S * bc)
        outs.append(xc)
    return outs


def _postprocess(R_list, base):
    """R per core [12, 2*bc] (pair-interleaved) -> full [B, 12] log_softmax."""
    outs = []
    for R in R_list:
        bc = R.shape[1] // 2
        Rs = (R[:, :bc] + R[:, bc:]).astype(np.float32)
        o = base[None, :] - (2.0 / S) * Rs.T
        m = o.max(axis=1, keepdims=True)
        z = o - m
        lse = np.log(np.exp(z).sum(axis=1, keepdims=True))
        outs.append(z - lse)
    return np.concatenate(outs, axis=0).astype(np.float32)


def _ensure_ntff_hook():
    """Inject antenv.axon_hooks (NTFF profile hook) if the image lacks it."""
    import sys
    try:
        from antenv.axon_hooks import get_axon_ntff_profile_hook  # noqa: F401
        return True
    except ImportError:
        pass
    import contextlib
    import ctypes
    import types

    so_path = "/opt/axon/libaxon_pjrt.so"
    try:
        lib = ctypes.CDLL(so_path)
    except OSError:
        return False
    if not hasattr(lib, "axon_start_nrt_profile"):
        return False
    lib.axon_start_nrt_profile.argtypes = [
        ctypes.POINTER(ctypes.c_int64),
        ctypes.c_size_t,
    ]
    lib.axon_start_nrt_profile.restype = ctypes.c_int64
    lib.axon_stop_nrt_profile.argtypes = [ctypes.c_char_p]
    lib.axon_stop_nrt_profile.restype = ctypes.c_int64

    @contextlib.contextmanager
    def _hook(output_dir, device_ids):
        import jax

        jax.devices()
        if device_ids:
            ids = (ctypes.c_int64 * len(device_ids))(*device_ids)
            rc = lib.axon_start_nrt_profile(ids, len(device_ids))
        else:
            rc = lib.axon_start_nrt_profile(None, 0)
        if rc != 0:
            raise RuntimeError(f"axon_start_nrt_profile rc={rc}")
        try:
            yield
        finally:
            n = lib.axon_stop_nrt_profile(str(output_dir).encode())
            if n < 0:
                raise RuntimeError(f"axon_stop_nrt_profile rc={n}")

    mod = types.ModuleType("antenv.axon_hooks")
    mod._hook = _hook
    mod.get_axon_ntff_profile_hook = lambda: _hook
    mod.set_axon_ntff_profile_hook = lambda h: setattr(mod, "_hook", h)
    import antenv

    sys.modules["antenv.axon_hooks"] = mod
    antenv.axon_hooks = mod
    return True


def kernel(x, W1, b1, W2, b2, W3, b3, Wr, br, _trace=False):
    from concourse.bass_utils import run_bass_kernel_spmd

    if _trace:
        _trace = _ensure_ntff_hook()
    nc = _get_nc()
    wmap, base = _prep_weights(W1, b1, W2, b2, W3, b3, Wr, br)
    xs = _prep_x(x)
    in_maps = [{**wmap, "x": xs[i]} for i in range(NCORES)]
    res = run_bass_kernel_spmd(
        nc, in_maps, core_ids=list(range(NCORES)), trace=_trace
    )
    R_list = [res.results[i]["out"] for i in range(NCORES)]
    out = _postprocess(R_list, base)
    if _trace:
        kernel.last_exec_time_ns = res.exec_time_ns
        kernel.last_results = res
    return out


kernel.last_exec_time_ns = None
kernel.last_results = None


# revision 10
# speedup vs baseline: 1.1364x; 1.1364x over previous
"""Trainium2 Bass kernel for the GSC Vanilla SNN problem.

3-layer LIF spiking net, S=101 timesteps, B=2048 batch, data-parallel over
batch across 8 NeuronCores (256 rows per core).

Math (per layer, per step, spikingjelly LIF with tau=2, v_th=1, hard reset):
    a_t   = v_{t-1} + c_t              (c_t = matmul current incl. bias)
    z_t   = 0.5 * (a_t < 2)            in {0, 0.5}
    v_t   = a_t * z_t                  (= a/2 if no spike else 0)
The true spike s = 1 - 2*z is folded into the next layer's weights
(s @ W + b == z @ (-2W) + (b + colsum(W))); biases ride the matmuls as extra
contraction rows (x gets two ones-rows; layers 2/3 use the always-0.5 pad
lanes h=200/201). Readout accumulates R = sum_t z3_t @ Wr_pad in PSUM
(pair-interleaved [12, 512]); host applies base - (2/S)(R0+R1), log_softmax.

Schedule: layer l is SKEWED by 2*l timesteps, so at any "mega-step" j the
three layers process independent steps (t_l = j - 2l). Their elementwise ops
merge into single wide DVE instructions over layer-major supertiles
([128, 3, 2, 256]), amortizing instruction overhead, and the PE always has a
full pair of matmul work whose inputs were produced a mega-pair earlier.
Timesteps are processed in PAIRS so every matmul runs at N=512 (a full PSUM
bank per m-chunk) and weight loads amortize.

Engines: PE does all matmuls (bf16, f32 PSUM) and, on even steps, also adds
the membrane state into the PSUM current via an identity matmul (so the ACT
evacuation directly yields a = v + c and the DVE add is skipped); ACT
evacuates each pair's PSUM to bf16 SBUF in one [128,1024] activation per
layer; DVE does the remaining adds (TT 2x bf16), the z compares (TS 4x), and
the reset multiplies (TT 2x).
"""

import numpy as np
import ml_dtypes

S = 101
D = 120            # C*M input features
DA = 122           # augmented with 2 ones-rows for hi/lo bias
H = 200
HP = 256           # padded hidden
DOUT = 12
NCORES = 8
B_FULL = 2048
BC = B_FULL // NCORES   # 256 batch rows per core
TB = 8                  # x DMA block (timesteps per DMA)
NL = 3                  # layers
SKEW = 2                # timestep skew between adjacent layers

CFG = {"pe_vadd": False}

_bf16 = ml_dtypes.bfloat16

_BUILD_CACHE = {}


def _build(s_steps, bc, tb, cfg=None):
    """Build + compile the Bass program for one core. Returns nc."""
    import concourse.bacc as bacc
    import concourse.mybir as mybir
    import concourse.tile as tile

    cfg = cfg or CFG
    pe_vadd = cfg.get("pe_vadd", True)
    dt = mybir.dt
    alu = mybir.AluOpType
    P = 128
    B2 = 2 * bc
    n_rpairs = (s_steps + 1) // 2          # real pairs per layer (last may be single)
    n_mpairs = n_rpairs + NL - 1           # mega-pairs

    def pair_len(q):  # steps in real pair q
        return min(2, s_steps - 2 * q)

    nc = bacc.Bacc("TRN2", target_bir_lowering=False, debug=False)

    x_d = nc.dram_tensor("x", [DA, s_steps * bc], dt.bfloat16, kind="ExternalInput")
    w1_d = nc.dram_tensor("w1", [DA, HP], dt.bfloat16, kind="ExternalInput")
    w2_d = nc.dram_tensor("w2", [2, P, HP], dt.bfloat16, kind="ExternalInput")
    w3_d = nc.dram_tensor("w3", [2, P, HP], dt.bfloat16, kind="ExternalInput")
    wr_d = nc.dram_tensor("wr", [2, P, DOUT], dt.bfloat16, kind="ExternalInput")
    eye_d = nc.dram_tensor("eye", [P, P], dt.bfloat16, kind="ExternalInput")
    out_d = nc.dram_tensor("out", [DOUT, B2], dt.float32, kind="ExternalOutput")

    with tile.TileContext(nc) as tc:
        with (
            tc.tile_pool(name="const", bufs=1) as constp,
            tc.tile_pool(name="xp", bufs=3) as xp,
            tc.tile_pool(name="state", bufs=1) as statep,
            tc.tile_pool(name="chp_", bufs=3) as chp_,
            tc.tile_pool(name="zp_", bufs=3) as zp_,
            tc.tile_pool(name="ap_", bufs=3) as ap_,
            tc.tile_pool(name="outp", bufs=1) as outp,
            tc.tile_pool(name="ps", bufs=1, space="PSUM") as psp,
            tc.tile_pool(name="psr", bufs=1, space="PSUM") as psrp,
        ):
            w1 = constp.tile([DA, HP], dt.bfloat16)
            nc.sync.dma_start(w1[:], w1_d[:])
            w2a = constp.tile([P, HP], dt.bfloat16)
            w2b = constp.tile([P, HP], dt.bfloat16)
            nc.sync.dma_start(w2a[:], w2_d[0])
            nc.sync.dma_start(w2b[:], w2_d[1])
            w3a = constp.tile([P, HP], dt.bfloat16)
            w3b = constp.tile([P, HP], dt.bfloat16)
            nc.sync.dma_start(w3a[:], w3_d[0])
            nc.sync.dma_start(w3b[:], w3_d[1])
            wra = constp.tile([P, DOUT], dt.bfloat16)
            wrb = constp.tile([P, DOUT], dt.bfloat16)
            nc.sync.dma_start(wra[:], wr_d[0])
            nc.sync.dma_start(wrb[:], wr_d[1])
            eye = constp.tile([P, P], dt.bfloat16)
            nc.sync.dma_start(eye[:], eye_d[:])

            # persistent membrane state, layer-major: [128, l, m, b]
            st = statep.tile([P, NL, 2, bc], dt.bfloat16)
            nc.vector.memset(st[:], 0.0)

            R = psrp.tile([DOUT, B2], dt.float32)

            w23 = [(w2a, w2b), (w3a, w3b)]
            # per-layer PSUM current pair tiles, created per mega-pair
            xb = None
            zprev_tiles = [None, None]  # z supertiles of mega-pairs J-1, J
            ch_cur = None

            for J in range(n_mpairs):
                # ---- matmul phase: layer l works real pair q = J - l ----
                lcs = [l for l in range(NL) if 0 <= J - l < n_rpairs]
                ch_prev = ch_cur
                ch_cur = chp_.tile([P, NL, 2, 2, bc], dt.bfloat16, name="ch")
                z_cur = zp_.tile([P, NL, 2, 2, bc], dt.bfloat16, name="z")
                c_tiles = {}
                for l in lcs:
                    q = J - l
                    np_ = pair_len(q)
                    W = np_ * bc
                    c = psp.tile([P, 2, B2], dt.float32, name=f"c{l + 1}")
                    c_tiles[l] = (c, q, np_, W)
                    if l == 0:
                        if (2 * q) % tb == 0:
                            t0 = 2 * q
                            ncols = min(tb, s_steps - t0) * bc
                            xb = xp.tile([DA, tb * bc], dt.bfloat16, name="xb")
                            nc.sync.dma_start(
                                xb[:, 0:ncols], x_d[:, t0 * bc : t0 * bc + ncols]
                            )
                        xpair = xb[:, (2 * q % tb) * bc : (2 * q % tb) * bc + W]
                        for m in range(2):
                            nc.tensor.matmul(
                                c[:, m, 0:W], w1[:, m * P : (m + 1) * P], xpair,
                                start=True, stop=not pe_vadd,
                                skip_group_check=True,
                            )
                    else:
                        zsrc = zprev_tiles[-1]  # written during mega-pair J-1
                        wka, wkb = w23[l - 1]
                        for m in range(2):
                            nc.tensor.matmul(
                                c[:, m, 0:W],
                                wka[:, m * P : (m + 1) * P],
                                zsrc[:, l - 1, 0, 0:np_, :],
                                start=True, stop=False,
                                skip_group_check=True,
                            )
                            nc.tensor.matmul(
                                c[:, m, 0:W],
                                wkb[:, m * P : (m + 1) * P],
                                zsrc[:, l - 1, 1, 0:np_, :],
                                start=False, stop=not pe_vadd,
                                skip_group_check=True,
                            )
                    if pe_vadd:
                        # add membrane state into the even-step half of the bank
                        for m in range(2):
                            nc.tensor.matmul(
                                c[:, m, 0:bc], eye[:], st[:, l, m, :],
                                start=False, stop=True,
                                skip_group_check=True,
                            )
                    # evacuate the pair's current to bf16 SBUF in one ACT op
                    nc.scalar.copy(
                        ch_cur[:, l, :, 0:np_, :],
                        c[:, :, 0:W].rearrange("p m (s b) -> p m s b", s=np_),
                    )

                # ---- readout: z3 of real pair q3 = J - NL + 1 ... wait, l=2 ----
                if zprev_tiles[-1] is not None:
                    q3 = (J - 1) - 2
                    if 0 <= q3 < n_rpairs:
                        np3 = pair_len(q3)
                        W3_ = np3 * bc
                        zsrc = zprev_tiles[-1]
                        nc.tensor.matmul(
                            R[:, 0:W3_], wra[:], zsrc[:, 2, 0, 0:np3, :],
                            start=(q3 == 0), stop=False, skip_group_check=True,
                        )
                        nc.tensor.matmul(
                            R[:, 0:W3_], wrb[:], zsrc[:, 2, 1, 0:np3, :],
                            start=False, stop=(q3 == n_rpairs - 1),
                            skip_group_check=True,
                        )

                # ---- elementwise phase: mega-steps 2J, 2J+1 ----
                for p in range(2):
                    j = 2 * J + p
                    lr = [l for l in range(NL)
                          if 0 <= j - SKEW * l < s_steps and (J - l) >= 0
                          and (J - l) < n_rpairs and p < pair_len(J - l)]
                    if not lr:
                        continue
                    l0, l1 = lr[0], lr[-1] + 1
                    L = l1 - l0
                    ch_sl = ch_cur[:, l0:l1, :, p, :]       # [128, L, 2, 256]
                    z_sl = z_cur[:, l0:l1, :, p, :]
                    st_sl = st[:, l0:l1, :, :]
                    if p == 0 and pe_vadd:
                        a_sl = ch_sl                         # a already = v + c
                    else:
                        a = ap_.tile([P, NL, 2, bc], dt.bfloat16, name="a")
                        a_sl = a[:, l0:l1, :, :]
                        nc.vector.tensor_tensor(a_sl, st_sl, ch_sl, op=alu.add)
                    nc.vector.tensor_scalar(
                        z_sl, a_sl, 2.0, 0.5, alu.is_lt, alu.mult
                    )
                    nc.vector.tensor_tensor(st_sl, a_sl, z_sl, op=alu.mult)

                zprev_tiles = [zprev_tiles[-1], z_cur]

            # ---- final readout for the last NL-1 mega-pairs' z3 ----
            for J in (n_mpairs, n_mpairs + 1):
                q3 = (J - 1) - 2
                if 0 <= q3 < n_rpairs:
                    np3 = pair_len(q3)
                    W3_ = np3 * bc
                    zsrc = zprev_tiles[-1] if J == n_mpairs else None
                    if zsrc is None:
                        break
                    nc.tensor.matmul(
                        R[:, 0:W3_], wra[:], zsrc[:, 2, 0, 0:np3, :],
                        start=(q3 == 0), stop=False, skip_group_check=True,
                    )
                    nc.tensor.matmul(
                        R[:, 0:W3_], wrb[:], zsrc[:, 2, 1, 0:np3, :],
                        start=False, stop=(q3 == n_rpairs - 1),
                        skip_group_check=True,
                    )

            out_sb = outp.tile([DOUT, B2], dt.float32)
            nc.vector.tensor_copy(out_sb[:], R[:])
            nc.sync.dma_start(out_d[:], out_sb[:])

    nc.compile()
    return nc


def _get_nc(s_steps=S, bc=BC, tb=TB):
    key = (s_steps, bc, tb)
    if key not in _BUILD_CACHE:
        _BUILD_CACHE[key] = _build(s_steps, bc, tb)
    return _BUILD_CACHE[key]


def _hi_lo(v):
    hi = v.astype(_bf16)
    lo = (v - hi.astype(np.float64)).astype(_bf16)
    return hi, lo


def _prep_weights(W1, b1, W2, b2, W3, b3, Wr, br):
    """Host-side weight packing. Returns (device array dict, host affine base)."""
    P = 128

    def pad(w, rows, cols, scale=1.0):
        w = np.asarray(w, np.float64) * scale
        out = np.zeros((rows, cols), np.float64)
        out[: w.shape[0], : w.shape[1]] = w
        return out

    w1p = pad(W1, DA, HP)
    bh = np.zeros(HP, np.float64)
    bh[:H] = np.asarray(b1, np.float64)
    w1p_bf = w1p.astype(_bf16)
    w1p_bf[D], w1p_bf[D + 1] = _hi_lo(bh)

    def mid(W, b):
        wp = pad(W, HP, HP, scale=-2.0).astype(_bf16)
        bh = np.zeros(HP, np.float64)
        bh[:H] = np.asarray(b, np.float64) + np.asarray(W, np.float64).sum(axis=0)
        hi, lo = _hi_lo(bh)
        wp[H] = 2.0 * hi.astype(np.float64)
        wp[H + 1] = 2.0 * lo.astype(np.float64)
        return wp.reshape(2, P, HP)

    w2p = mid(W2, b2)
    w3p = mid(W3, b3)
    wrp = pad(Wr, HP, DOUT).astype(_bf16).reshape(2, P, DOUT)

    base = (np.asarray(br, np.float64) + np.asarray(Wr, np.float64).sum(axis=0)).astype(
        np.float32
    )
    eye = np.eye(P, dtype=_bf16)
    return {"w1": w1p_bf, "w2": w2p, "w3": w3p, "wr": wrp, "eye": eye}, base


def _prep_x(x):
    """[B,C,S,M] f32 -> per-core [DA, S*bc] bf16 list (with two ones-rows)."""
    x = np.asarray(x, np.float32)
    B = x.shape[0]
    bc = B // NCORES
    xt = np.ascontiguousarray(x.transpose(1, 3, 2, 0)).reshape(D, S, B).astype(_bf16)
    outs = []
    for i in range(NCORES):
        xc = np.ones((DA, S * bc), dtype=_bf16)
        xc[:D] = xt[:, :, i * bc : (i + 1) * bc].reshape(D, S * bc)
        outs.append(xc)
    return outs


def _postprocess(R_list, base):
    """R per core [12, 2*bc] (pair-interleaved) -> full [B, 12] log_softmax."""
    outs = []
    for R in R_list:
        bc = R.shape[1] // 2
        Rs = (R[:, :bc] + R[:, bc:]).astype(np.float32)
        o = base[None, :] - (2.0 / S) * Rs.T
        m = o.max(axis=1, keepdims=True)
        z = o - m
        lse = np.log(np.exp(z).sum(axis=1, keepdims=True))
        outs.append(z - lse)
    return np.concatenate(outs, axis=0).astype(np.float32)


def _ensure_ntff_hook():
    """Inject antenv.axon_hooks (NTFF profile hook) if the image lacks it."""
    import sys
    try:
        from antenv.axon_hooks import get_axon_ntff_profile_hook  # noqa: F401
        return True
    except ImportError:
        pass
    import contextlib
    import ctypes
    import types

    so_path = "/opt/axon/libaxon_pjrt.so"
    try:
        lib = ctypes.CDLL(so_path)
    except OSError:
        return False
    if not hasattr(lib, "axon_start_nrt_profile"):
        return False
    lib.axon_start_nrt_profile.argtypes = [
        ctypes.POINTER(ctypes.c_int64),
        ctypes.c_size_t,
    ]
    lib.axon_start_nrt_profile.restype = ctypes.c_int64
    lib.axon_stop_nrt_profile.argtypes = [ctypes.c_char_p]
    lib.axon_stop_nrt_profile.restype = ctypes.c_int64

    @contextlib.contextmanager
    def _hook(output_dir, device_ids):
        import jax

        jax.devices()
        if device_ids:
            ids = (ctypes.c_int64 * len(device_ids))(*device_ids)
            rc = lib.axon_start_nrt_profile(ids, len(device_ids))
        else:
            rc = lib.axon_start_nrt_profile(None, 0)
        if rc != 0:
            raise RuntimeError(f"axon_start_nrt_profile rc={rc}")
        try:
            yield
        finally:
            n = lib.axon_stop_nrt_profile(str(output_dir).encode())
            if n < 0:
                raise RuntimeError(f"axon_stop_nrt_profile rc={n}")

    mod = types.ModuleType("antenv.axon_hooks")
    mod._hook = _hook
    mod.get_axon_ntff_profile_hook = lambda: _hook
    mod.set_axon_ntff_profile_hook = lambda h: setattr(mod, "_hook", h)
    import antenv

    sys.modules["antenv.axon_hooks"] = mod
    antenv.axon_hooks = mod
    return True


def kernel(x, W1, b1, W2, b2, W3, b3, Wr, br, _trace=False):
    from concourse.bass_utils import run_bass_kernel_spmd

    if _trace:
        _trace = _ensure_ntff_hook()
    nc = _get_nc()
    wmap, base = _prep_weights(W1, b1, W2, b2, W3, b3, Wr, br)
    xs = _prep_x(x)
    in_maps = [{**wmap, "x": xs[i]} for i in range(NCORES)]
    res = run_bass_kernel_spmd(
        nc, in_maps, core_ids=list(range(NCORES)), trace=_trace
    )
    R_list = [res.results[i]["out"] for i in range(NCORES)]
    out = _postprocess(R_list, base)
    if _trace:
        kernel.last_exec_time_ns = res.exec_time_ns
        kernel.last_results = res
    return out


kernel.last_exec_time_ns = None
kernel.last_results = None


# revision 12
# speedup vs baseline: 1.7256x; 1.5185x over previous
"""Trainium2 Bass kernel for the GSC Vanilla SNN problem.

3-layer LIF spiking net, S=101 timesteps, B=2048 batch, data-parallel over
batch across 8 NeuronCores (256 rows per core).

Math (per layer, per step, spikingjelly LIF with tau=2, v_th=1, hard reset):
    a_t   = v_{t-1} + c_t              (c_t = matmul current incl. bias)
    z_t   = 0.5 * (a_t < 2)            in {0, 0.5}
    v_t   = a_t * z_t                  (= a/2 if no spike else 0)
The true spike s = 1 - 2*z is folded into the next layer's weights
(s @ W + b == z @ (-2W) + (b + colsum(W))); biases ride the matmuls as extra
contraction rows (x gets two ones-rows; layers 2/3 use the always-0.5 pad
lanes h=200/201). Readout accumulates R = sum_t z3_t @ Wr_pad in PSUM
(pair-interleaved [12, 512]); host applies base - (2/S)(R0+R1), log_softmax.

Schedule: layer l is SKEWED by 2*l timesteps, so at any "mega-step" j the
three layers process independent steps (t_l = j - 2l). Their elementwise ops
merge into single wide DVE instructions over layer-major supertiles
([128, 3, 2, 256]), amortizing instruction overhead, and the PE always has a
full pair of matmul work whose inputs were produced a mega-pair earlier.
Timesteps are processed in PAIRS so every matmul runs at N=512 (a full PSUM
bank per m-chunk) and weight loads amortize.

Engines: PE does all matmuls (bf16, f32 PSUM) and, on even steps, also adds
the membrane state into the PSUM current via an identity matmul (so the ACT
evacuation directly yields a = v + c and the DVE add is skipped); ACT
evacuates each pair's PSUM to bf16 SBUF in one [128,1024] activation per
layer; DVE does the remaining adds (TT 2x bf16), the z compares (TS 4x), and
the reset multiplies (TT 2x).
"""

import numpy as np
import ml_dtypes

S = 101
D = 120            # C*M input features
DA = 122           # augmented with 2 ones-rows for hi/lo bias
H = 200
HP = 256           # padded hidden
DOUT = 12
NCORES = 8
B_FULL = 2048
BC = B_FULL // NCORES   # 256 batch rows per core
TB = 8                  # x DMA block (timesteps per DMA)
NL = 3                  # layers
SKEW = 2                # timestep skew between adjacent layers

CFG = {"pe_vadd": False}

_bf16 = ml_dtypes.bfloat16

_BUILD_CACHE = {}


def _build(s_steps, bc, tb, cfg=None):
    """Build + compile the Bass program for one core. Returns nc."""
    import concourse.bacc as bacc
    import concourse.mybir as mybir
    import concourse.tile as tile

    cfg = cfg or CFG
    pe_vadd = cfg.get("pe_vadd", True)
    dt = mybir.dt
    alu = mybir.AluOpType
    P = 128
    B2 = 2 * bc
    n_rpairs = (s_steps + 1) // 2          # real pairs per layer (last may be single)
    n_mpairs = n_rpairs + NL - 1           # mega-pairs

    def pair_len(q):  # steps in real pair q
        return min(2, s_steps - 2 * q)

    nc = bacc.Bacc("TRN2", target_bir_lowering=False, debug=False)

    x_d = nc.dram_tensor("x", [DA, s_steps * bc], dt.bfloat16, kind="ExternalInput")
    w1_d = nc.dram_tensor("w1", [DA, HP], dt.bfloat16, kind="ExternalInput")
    w2_d = nc.dram_tensor("w2", [2, P, HP], dt.bfloat16, kind="ExternalInput")
    w3_d = nc.dram_tensor("w3", [2, P, HP], dt.bfloat16, kind="ExternalInput")
    wr_d = nc.dram_tensor("wr", [2, P, DOUT], dt.bfloat16, kind="ExternalInput")
    eye_d = nc.dram_tensor("eye", [P, P], dt.bfloat16, kind="ExternalInput")
    out_d = nc.dram_tensor("out", [DOUT, B2], dt.float32, kind="ExternalOutput")

    with tile.TileContext(nc) as tc:
        with (
            tc.tile_pool(name="const", bufs=1) as constp,
            tc.tile_pool(name="xp", bufs=3) as xp,
            tc.tile_pool(name="state", bufs=1) as statep,
            tc.tile_pool(name="chp_", bufs=3) as chp_,
            tc.tile_pool(name="zp_", bufs=3) as zp_,
            tc.tile_pool(name="ap_", bufs=3) as ap_,
            tc.tile_pool(name="outp", bufs=1) as outp,
            tc.tile_pool(name="ps", bufs=1, space="PSUM") as psp,
            tc.tile_pool(name="psr", bufs=1, space="PSUM") as psrp,
        ):
            w1 = constp.tile([DA, HP], dt.bfloat16)
            nc.sync.dma_start(w1[:], w1_d[:])
            w2a = constp.tile([P, HP], dt.bfloat16)
            w2b = constp.tile([P, HP], dt.bfloat16)
            nc.sync.dma_start(w2a[:], w2_d[0])
            nc.sync.dma_start(w2b[:], w2_d[1])
            w3a = constp.tile([P, HP], dt.bfloat16)
            w3b = constp.tile([P, HP], dt.bfloat16)
            nc.sync.dma_start(w3a[:], w3_d[0])
            nc.sync.dma_start(w3b[:], w3_d[1])
            wra = constp.tile([P, DOUT], dt.bfloat16)
            wrb = constp.tile([P, DOUT], dt.bfloat16)
            nc.sync.dma_start(wra[:], wr_d[0])
            nc.sync.dma_start(wrb[:], wr_d[1])
            eye = constp.tile([P, P], dt.bfloat16)
            nc.sync.dma_start(eye[:], eye_d[:])

            # persistent membrane state, layer-major: [128, l, m, b]
            st = statep.tile([P, NL, 2, bc], dt.bfloat16)
            nc.vector.memset(st[:], 0.0)

            R = psrp.tile([DOUT, B2], dt.float32)

            w23 = [(w2a, w2b), (w3a, w3b)]
            # per-layer PSUM current pair tiles, created per mega-pair
            xb = None
            zprev_tiles = [None, None]  # z supertiles of mega-pairs J-1, J
            ch_cur = None

            for J in range(n_mpairs):
                # ---- matmul phase: layer l works real pair q = J - l ----
                lcs = [l for l in range(NL) if 0 <= J - l < n_rpairs]
                ch_prev = ch_cur
                ch_cur = chp_.tile([P, NL, 2, 2, bc], dt.bfloat16, name="ch")
                z_cur = zp_.tile([P, NL, 2, 2, bc], dt.bfloat16, name="z")
                c_tiles = {}
                for l in lcs:
                    q = J - l
                    np_ = pair_len(q)
                    W = np_ * bc
                    c = psp.tile([P, 2, B2], dt.float32, name=f"c{l + 1}")
                    c_tiles[l] = (c, q, np_, W)
                    if l == 0:
                        if (2 * q) % tb == 0:
                            t0 = 2 * q
                            ncols = min(tb, s_steps - t0) * bc
                            xb = xp.tile([DA, tb * bc], dt.bfloat16, name="xb")
                            nc.sync.dma_start(
                                xb[:, 0:ncols], x_d[:, t0 * bc : t0 * bc + ncols]
                            )
                        xpair = xb[:, (2 * q % tb) * bc : (2 * q % tb) * bc + W]
                        for m in range(2):
                            nc.tensor.matmul(
                                c[:, m, 0:W], w1[:, m * P : (m + 1) * P], xpair,
                                start=True, stop=not pe_vadd,
                                skip_group_check=True,
                            )
                    else:
                        zsrc = zprev_tiles[-1]  # written during mega-pair J-1
                        wka, wkb = w23[l - 1]
                        for m in range(2):
                            nc.tensor.matmul(
                                c[:, m, 0:W],
                                wka[:, m * P : (m + 1) * P],
                                zsrc[:, l - 1, 0, 0:np_, :],
                                start=True, stop=False,
                                skip_group_check=True,
                            )
                            nc.tensor.matmul(
                                c[:, m, 0:W],
                                wkb[:, m * P : (m + 1) * P],
                                zsrc[:, l - 1, 1, 0:np_, :],
                                start=False, stop=not pe_vadd,
                                skip_group_check=True,
                            )
                    if pe_vadd:
                        # add membrane state into the even-step half of the bank
                        for m in range(2):
                            nc.tensor.matmul(
                                c[:, m, 0:bc], eye[:], st[:, l, m, :],
                                start=False, stop=True,
                                skip_group_check=True,
                            )
                    # evacuate the pair's current to bf16 SBUF in one ACT op
                    nc.scalar.copy(
                        ch_cur[:, l, :, 0:np_, :],
                        c[:, :, 0:W].rearrange("p m (s b) -> p m s b", s=np_),
                    )

                # ---- readout: z3 of real pair q3 = J - NL + 1 ... wait, l=2 ----
                if zprev_tiles[-1] is not None:
                    q3 = (J - 1) - 2
                    if 0 <= q3 < n_rpairs:
                        np3 = pair_len(q3)
                        W3_ = np3 * bc
                        zsrc = zprev_tiles[-1]
                        nc.tensor.matmul(
                            R[:, 0:W3_], wra[:], zsrc[:, 2, 0, 0:np3, :],
                            start=(q3 == 0), stop=False, skip_group_check=True,
                        )
                        nc.tensor.matmul(
                            R[:, 0:W3_], wrb[:], zsrc[:, 2, 1, 0:np3, :],
                            start=False, stop=(q3 == n_rpairs - 1),
                            skip_group_check=True,
                        )

                # ---- elementwise phase: mega-steps 2J, 2J+1 ----
                for p in range(2):
                    j = 2 * J + p
                    lr = [l for l in range(NL)
                          if 0 <= j - SKEW * l < s_steps and (J - l) >= 0
                          and (J - l) < n_rpairs and p < pair_len(J - l)]
                    if not lr:
                        continue
                    # per-layer ops (fine-grained deps); adjacent ops in each
                    # group are independent so DVE pipelines their overheads
                    a = ap_.tile([P, NL, 2, bc], dt.bfloat16, name="a")
                    a_sls = {}
                    for l in lr:
                        ch_sl = ch_cur[:, l, :, p, :]
                        if p == 0 and pe_vadd:
                            a_sls[l] = ch_sl
                        else:
                            a_sls[l] = a[:, l, :, :]
                            nc.vector.tensor_tensor(
                                a_sls[l], st[:, l, :, :], ch_sl, op=alu.add
                            )
                    for l in lr:
                        nc.vector.tensor_scalar(
                            z_cur[:, l, :, p, :], a_sls[l], 2.0, 0.5,
                            alu.is_lt, alu.mult,
                        )
                    for l in lr:
                        nc.vector.tensor_tensor(
                            st[:, l, :, :], a_sls[l], z_cur[:, l, :, p, :],
                            op=alu.mult,
                        )

                zprev_tiles = [zprev_tiles[-1], z_cur]

            # ---- final readout for the last NL-1 mega-pairs' z3 ----
            for J in (n_mpairs, n_mpairs + 1):
                q3 = (J - 1) - 2
                if 0 <= q3 < n_rpairs:
                    np3 = pair_len(q3)
                    W3_ = np3 * bc
                    zsrc = zprev_tiles[-1] if J == n_mpairs else None
                    if zsrc is None:
                        break
                    nc.tensor.matmul(
                        R[:, 0:W3_], wra[:], zsrc[:, 2, 0, 0:np3, :],
                        start=(q3 == 0), stop=False, skip_group_check=True,
                    )
                    nc.tensor.matmul(
                        R[:, 0:W3_], wrb[:], zsrc[:, 2, 1, 0:np3, :],
                        start=False, stop=(q3 == n_rpairs - 1),
                        skip_group_check=True,
                    )

            out_sb = outp.tile([DOUT, B2], dt.float32)
            nc.vector.tensor_copy(out_sb[:], R[:])
            nc.sync.dma_start(out_d[:], out_sb[:])

    nc.compile()
    return nc


def _get_nc(s_steps=S, bc=BC, tb=TB):
    key = (s_steps, bc, tb)
    if key not in _BUILD_CACHE:
        _BUILD_CACHE[key] = _build(s_steps, bc, tb)
    return _BUILD_CACHE[key]


def _hi_lo(v):
    hi = v.astype(_bf16)
    lo = (v - hi.astype(np.float64)).astype(_bf16)
    return hi, lo


def _prep_weights(W1, b1, W2, b2, W3, b3, Wr, br):
    """Host-side weight packing. Returns (device array dict, host affine base)."""
    P = 128

    def pad(w, rows, cols, scale=1.0):
        w = np.asarray(w, np.float64) * scale
        out = np.zeros((rows, cols), np.float64)
        out[: w.shape[0], : w.shape[1]] = w
        return out

    w1p = pad(W1, DA, HP)
    bh = np.zeros(HP, np.float64)
    bh[:H] = np.asarray(b1, np.float64)
    w1p_bf = w1p.astype(_bf16)
    w1p_bf[D], w1p_bf[D + 1] = _hi_lo(bh)

    def mid(W, b):
        wp = pad(W, HP, HP, scale=-2.0).astype(_bf16)
        bh = np.zeros(HP, np.float64)
        bh[:H] = np.asarray(b, np.float64) + np.asarray(W, np.float64).sum(axis=0)
        hi, lo = _hi_lo(bh)
        wp[H] = 2.0 * hi.astype(np.float64)
        wp[H + 1] = 2.0 * lo.astype(np.float64)
        return wp.reshape(2, P, HP)

    w2p = mid(W2, b2)
    w3p = mid(W3, b3)
    wrp = pad(Wr, HP, DOUT).astype(_bf16).reshape(2, P, DOUT)

    base = (np.asarray(br, np.float64) + np.asarray(Wr, np.float64).sum(axis=0)).astype(
        np.float32
    )
    eye = np.eye(P, dtype=_bf16)
    return {"w1": w1p_bf, "w2": w2p, "w3": w3p, "wr": wrp, "eye": eye}, base


def _prep_x(x):
    """[B,C,S,M] f32 -> per-core [DA, S*bc] bf16 list (with two ones-rows)."""
    x = np.asarray(x, np.float32)
    B = x.shape[0]
    bc = B // NCORES
    xt = np.ascontiguousarray(x.transpose(1, 3, 2, 0)).reshape(D, S, B).astype(_bf16)
    outs = []
    for i in range(NCORES):
        xc = np.ones((DA, S * bc), dtype=_bf16)
        xc[:D] = xt[:, :, i * bc : (i + 1) * bc].reshape(D, S * bc)
        outs.append(xc)
    return outs


def _postprocess(R_list, base):
    """R per core [12, 2*bc] (pair-interleaved) -> full [B, 12] log_softmax."""
    outs = []
    for R in R_list:
        bc = R.shape[1] // 2
        Rs = (R[:, :bc] + R[:, bc:]).astype(np.float32)
        o = base[None, :] - (2.0 / S) * Rs.T
        m = o.max(axis=1, keepdims=True)
        z = o - m
        lse = np.log(np.exp(z).sum(axis=1, keepdims=True))
        outs.append(z - lse)
    return np.concatenate(outs, axis=0).astype(np.float32)


def _ensure_ntff_hook():
    """Inject antenv.axon_hooks (NTFF profile hook) if the image lacks it."""
    import sys
    try:
        from antenv.axon_hooks import get_axon_ntff_profile_hook  # noqa: F401
        return True
    except ImportError:
        pass
    import contextlib
    import ctypes
    import types

    so_path = "/opt/axon/libaxon_pjrt.so"
    try:
        lib = ctypes.CDLL(so_path)
    except OSError:
        return False
    if not hasattr(lib, "axon_start_nrt_profile"):
        return False
    lib.axon_start_nrt_profile.argtypes = [
        ctypes.POINTER(ctypes.c_int64),
        ctypes.c_size_t,
    ]
    lib.axon_start_nrt_profile.restype = ctypes.c_int64
    lib.axon_stop_nrt_profile.argtypes = [ctypes.c_char_p]
    lib.axon_stop_nrt_profile.restype = ctypes.c_int64

    @contextlib.contextmanager
    def _hook(output_dir, device_ids):
        import jax

        jax.devices()
        if device_ids:
            ids = (ctypes.c_int64 * len(device_ids))(*device_ids)
            rc = lib.axon_start_nrt_profile(ids, len(device_ids))
        else:
            rc = lib.axon_start_nrt_profile(None, 0)
        if rc != 0:
            raise RuntimeError(f"axon_start_nrt_profile rc={rc}")
        try:
            yield
        finally:
            n = lib.axon_stop_nrt_profile(str(output_dir).encode())
            if n < 0:
                raise RuntimeError(f"axon_stop_nrt_profile rc={n}")

    mod = types.ModuleType("antenv.axon_hooks")
    mod._hook = _hook
    mod.get_axon_ntff_profile_hook = lambda: _hook
    mod.set_axon_ntff_profile_hook = lambda h: setattr(mod, "_hook", h)
    import antenv

    sys.modules["antenv.axon_hooks"] = mod
    antenv.axon_hooks = mod
    return True


def kernel(x, W1, b1, W2, b2, W3, b3, Wr, br, _trace=False):
    from concourse.bass_utils import run_bass_kernel_spmd

    if _trace:
        _trace = _ensure_ntff_hook()
    nc = _get_nc()
    wmap, base = _prep_weights(W1, b1, W2, b2, W3, b3, Wr, br)
    xs = _prep_x(x)
    in_maps = [{**wmap, "x": xs[i]} for i in range(NCORES)]
    res = run_bass_kernel_spmd(
        nc, in_maps, core_ids=list(range(NCORES)), trace=_trace
    )
    R_list = [res.results[i]["out"] for i in range(NCORES)]
    out = _postprocess(R_list, base)
    if _trace:
        kernel.last_exec_time_ns = res.exec_time_ns
        kernel.last_results = res
    return out


kernel.last_exec_time_ns = None
kernel.last_results = None


# revision 13
# speedup vs baseline: 1.7265x; 1.0005x over previous
"""Trainium2 Bass kernel for the GSC Vanilla SNN problem.

3-layer LIF spiking net, S=101 timesteps, B=2048 batch, data-parallel over
batch across 8 NeuronCores (256 rows per core).

Math (per layer, per step, spikingjelly LIF with tau=2, v_th=1, hard reset):
    a_t   = v_{t-1} + c_t              (c_t = matmul current incl. bias)
    z_t   = 0.5 * (a_t < 2)            in {0, 0.5}
    v_t   = a_t * z_t                  (= a/2 if no spike else 0)
The true spike s = 1 - 2*z is folded into the next layer's weights
(s @ W + b == z @ (-2W) + (b + colsum(W))); biases ride the matmuls as extra
contraction rows (x gets two ones-rows; layers 2/3 use the always-0.5 pad
lanes h=200/201). Readout accumulates R = sum_t z3_t @ Wr_pad in PSUM
(pair-interleaved [12, 512]); host applies base - (2/S)(R0+R1), log_softmax.

Schedule: layer l is SKEWED by 2*l timesteps, so at any "mega-step" j the
three layers process independent steps (t_l = j - 2l). Their elementwise ops
merge into single wide DVE instructions over layer-major supertiles
([128, 3, 2, 256]), amortizing instruction overhead, and the PE always has a
full pair of matmul work whose inputs were produced a mega-pair earlier.
Timesteps are processed in PAIRS so every matmul runs at N=512 (a full PSUM
bank per m-chunk) and weight loads amortize.

Engines: PE does all matmuls (bf16, f32 PSUM) and, on even steps, also adds
the membrane state into the PSUM current via an identity matmul (so the ACT
evacuation directly yields a = v + c and the DVE add is skipped); ACT
evacuates each pair's PSUM to bf16 SBUF in one [128,1024] activation per
layer; DVE does the remaining adds (TT 2x bf16), the z compares (TS 4x), and
the reset multiplies (TT 2x).
"""

import numpy as np
import ml_dtypes

S = 101
D = 120            # C*M input features
DA = 122           # augmented with 2 ones-rows for hi/lo bias
H = 200
HP = 256           # padded hidden
DOUT = 12
NCORES = 8
B_FULL = 2048
BC = B_FULL // NCORES   # 256 batch rows per core
TB = 8                  # x DMA block (timesteps per DMA)
NL = 3                  # layers
SKEW = 2                # timestep skew between adjacent layers

CFG = {"pe_vadd": False}

_bf16 = ml_dtypes.bfloat16

_BUILD_CACHE = {}


def _build(s_steps, bc, tb, cfg=None):
    """Build + compile the Bass program for one core. Returns nc."""
    import concourse.bacc as bacc
    import concourse.mybir as mybir
    import concourse.tile as tile

    cfg = cfg or CFG
    pe_vadd = cfg.get("pe_vadd", True)
    dt = mybir.dt
    alu = mybir.AluOpType
    P = 128
    B2 = 2 * bc
    n_rpairs = (s_steps + 1) // 2          # real pairs per layer (last may be single)
    n_mpairs = n_rpairs + NL - 1           # mega-pairs

    def pair_len(q):  # steps in real pair q
        return min(2, s_steps - 2 * q)

    nc = bacc.Bacc("TRN2", target_bir_lowering=False, debug=False)

    x_d = nc.dram_tensor("x", [DA, s_steps * bc], dt.bfloat16, kind="ExternalInput")
    w1_d = nc.dram_tensor("w1", [DA, HP], dt.bfloat16, kind="ExternalInput")
    w2_d = nc.dram_tensor("w2", [2, P, HP], dt.bfloat16, kind="ExternalInput")
    w3_d = nc.dram_tensor("w3", [2, P, HP], dt.bfloat16, kind="ExternalInput")
    wr_d = nc.dram_tensor("wr", [2, P, DOUT], dt.bfloat16, kind="ExternalInput")
    eye_d = nc.dram_tensor("eye", [P, P], dt.bfloat16, kind="ExternalInput")
    out_d = nc.dram_tensor("out", [DOUT, B2], dt.float32, kind="ExternalOutput")

    with tile.TileContext(nc) as tc:
        with (
            tc.tile_pool(name="const", bufs=1) as constp,
            tc.tile_pool(name="xp", bufs=3) as xp,
            tc.tile_pool(name="state", bufs=1) as statep,
            tc.tile_pool(name="chp_", bufs=3) as chp_,
            tc.tile_pool(name="zp_", bufs=3) as zp_,
            tc.tile_pool(name="ap_", bufs=3) as ap_,
            tc.tile_pool(name="outp", bufs=1) as outp,
            tc.tile_pool(name="ps", bufs=1, space="PSUM") as psp,
            tc.tile_pool(name="psr", bufs=1, space="PSUM") as psrp,
        ):
            w1 = constp.tile([DA, HP], dt.bfloat16)
            nc.sync.dma_start(w1[:], w1_d[:])
            w2a = constp.tile([P, HP], dt.bfloat16)
            w2b = constp.tile([P, HP], dt.bfloat16)
            nc.sync.dma_start(w2a[:], w2_d[0])
            nc.sync.dma_start(w2b[:], w2_d[1])
            w3a = constp.tile([P, HP], dt.bfloat16)
            w3b = constp.tile([P, HP], dt.bfloat16)
            nc.sync.dma_start(w3a[:], w3_d[0])
            nc.sync.dma_start(w3b[:], w3_d[1])
            wra = constp.tile([P, DOUT], dt.bfloat16)
            wrb = constp.tile([P, DOUT], dt.bfloat16)
            nc.sync.dma_start(wra[:], wr_d[0])
            nc.sync.dma_start(wrb[:], wr_d[1])
            eye = constp.tile([P, P], dt.bfloat16)
            nc.sync.dma_start(eye[:], eye_d[:])

            # persistent membrane state, layer-major: [128, l, m, b]
            st = statep.tile([P, NL, 2, bc], dt.bfloat16)
            nc.vector.memset(st[:], 0.0)

            R = psrp.tile([DOUT, B2], dt.float32)

            w23 = [(w2a, w2b), (w3a, w3b)]
            # per-layer PSUM current pair tiles, created per mega-pair
            xb = None
            zprev_tiles = [None, None]  # z supertiles of mega-pairs J-1, J
            ch_cur = None

            for J in range(n_mpairs):
                # ---- matmul phase: layer l works real pair q = J - l ----
                lcs = [l for l in range(NL) if 0 <= J - l < n_rpairs]
                ch_prev = ch_cur
                ch_cur = chp_.tile([P, NL, 2, 2, bc], dt.bfloat16, name="ch")
                z_cur = zp_.tile([P, NL, 2, 2, bc], dt.bfloat16, name="z")
                c_tiles = {}
                for l in lcs:
                    q = J - l
                    np_ = pair_len(q)
                    W = np_ * bc
                    c = psp.tile([P, 2, B2], dt.float32, name=f"c{l + 1}")
                    c_tiles[l] = (c, q, np_, W)
                    if l == 0:
                        if (2 * q) % tb == 0:
                            t0 = 2 * q
                            ncols = min(tb, s_steps - t0) * bc
                            xb = xp.tile([DA, tb * bc], dt.bfloat16, name="xb")
                            nc.sync.dma_start(
                                xb[:, 0:ncols], x_d[:, t0 * bc : t0 * bc + ncols]
                            )
                        xpair = xb[:, (2 * q % tb) * bc : (2 * q % tb) * bc + W]
                        for m in range(2):
                            nc.tensor.matmul(
                                c[:, m, 0:W], w1[:, m * P : (m + 1) * P], xpair,
                                start=True, stop=not pe_vadd,
                                skip_group_check=True,
                            )
                    else:
                        zsrc = zprev_tiles[-1]  # written during mega-pair J-1
                        wka, wkb = w23[l - 1]
                        for m in range(2):
                            # m=1 swaps the K-chunk order so adjacent matmuls
                            # share the stationary weights (fewer LDWEIGHTS);
                            # the two PSUM addends commute exactly in f32.
                            order = ((wka, 0, True), (wkb, 1, False))
                            if m == 1:
                                order = ((wkb, 1, True), (wka, 0, False))
                            for wk, kk, first in order:
                                nc.tensor.matmul(
                                    c[:, m, 0:W],
                                    wk[:, m * P : (m + 1) * P],
                                    zsrc[:, l - 1, kk, 0:np_, :],
                                    start=first, stop=(not pe_vadd) and not first,
                                    skip_group_check=True,
                                )
                    if pe_vadd:
                        # add membrane state into the even-step half of the bank
                        for m in range(2):
                            nc.tensor.matmul(
                                c[:, m, 0:bc], eye[:], st[:, l, m, :],
                                start=False, stop=True,
                                skip_group_check=True,
                            )
                    # evacuate the pair's current to bf16 SBUF in one ACT op
                    nc.scalar.copy(
                        ch_cur[:, l, :, 0:np_, :],
                        c[:, :, 0:W].rearrange("p m (s b) -> p m s b", s=np_),
                    )

                # ---- readout: z3 of real pair q3 = J - NL + 1 ... wait, l=2 ----
                if zprev_tiles[-1] is not None:
                    q3 = (J - 1) - 2
                    if 0 <= q3 < n_rpairs:
                        np3 = pair_len(q3)
                        W3_ = np3 * bc
                        zsrc = zprev_tiles[-1]
                        nc.tensor.matmul(
                            R[:, 0:W3_], wra[:], zsrc[:, 2, 0, 0:np3, :],
                            start=(q3 == 0), stop=False, skip_group_check=True,
                        )
                        nc.tensor.matmul(
                            R[:, 0:W3_], wrb[:], zsrc[:, 2, 1, 0:np3, :],
                            start=False, stop=(q3 == n_rpairs - 1),
                            skip_group_check=True,
                        )

                # ---- elementwise phase: mega-steps 2J, 2J+1 ----
                for p in range(2):
                    j = 2 * J + p
                    lr = [l for l in range(NL)
                          if 0 <= j - SKEW * l < s_steps and (J - l) >= 0
                          and (J - l) < n_rpairs and p < pair_len(J - l)]
                    if not lr:
                        continue
                    # per-layer ops (fine-grained deps); adjacent ops in each
                    # group are independent so DVE pipelines their overheads
                    a = ap_.tile([P, NL, 2, bc], dt.bfloat16, name="a")
                    a_sls = {}
                    for l in lr:
                        ch_sl = ch_cur[:, l, :, p, :]
                        if p == 0 and pe_vadd:
                            a_sls[l] = ch_sl
                        else:
                            a_sls[l] = a[:, l, :, :]
                            nc.vector.tensor_tensor(
                                a_sls[l], st[:, l, :, :], ch_sl, op=alu.add
                            )
                    for l in lr:
                        nc.vector.tensor_scalar(
                            z_cur[:, l, :, p, :], a_sls[l], 2.0, 0.5,
                            alu.is_lt, alu.mult,
                        )
                    for l in lr:
                        nc.vector.tensor_tensor(
                            st[:, l, :, :], a_sls[l], z_cur[:, l, :, p, :],
                            op=alu.mult,
                        )

                zprev_tiles = [zprev_tiles[-1], z_cur]

            # ---- final readout for the last NL-1 mega-pairs' z3 ----
            for J in (n_mpairs, n_mpairs + 1):
                q3 = (J - 1) - 2
                if 0 <= q3 < n_rpairs:
                    np3 = pair_len(q3)
                    W3_ = np3 * bc
                    zsrc = zprev_tiles[-1] if J == n_mpairs else None
                    if zsrc is None:
                        break
                    nc.tensor.matmul(
                        R[:, 0:W3_], wra[:], zsrc[:, 2, 0, 0:np3, :],
                        start=(q3 == 0), stop=False, skip_group_check=True,
                    )
                    nc.tensor.matmul(
                        R[:, 0:W3_], wrb[:], zsrc[:, 2, 1, 0:np3, :],
                        start=False, stop=(q3 == n_rpairs - 1),
                        skip_group_check=True,
                    )

            out_sb = outp.tile([DOUT, B2], dt.float32)
            nc.vector.tensor_copy(out_sb[:], R[:])
            nc.sync.dma_start(out_d[:], out_sb[:])

    nc.compile()
    return nc


def _get_nc(s_steps=S, bc=BC, tb=TB):
    key = (s_steps, bc, tb)
    if key not in _BUILD_CACHE:
        _BUILD_CACHE[key] = _build(s_steps, bc, tb)
    return _BUILD_CACHE[key]


def _hi_lo(v):
    hi = v.astype(_bf16)
    lo = (v - hi.astype(np.float64)).astype(_bf16)
    return hi, lo


def _prep_weights(W1, b1, W2, b2, W3, b3, Wr, br):
    """Host-side weight packing. Returns (device array dict, host affine base)."""
    P = 128

    def pad(w, rows, cols, scale=1.0):
        w = np.asarray(w, np.float64) * scale
        out = np.zeros((rows, cols), np.float64)
        out[: w.shape[0], : w.shape[1]] = w
        return out

    w1p = pad(W1, DA, HP)
    bh = np.zeros(HP, np.float64)
    bh[:H] = np.asarray(b1, np.float64)
    w1p_bf = w1p.astype(_bf16)
    w1p_bf[D], w1p_bf[D + 1] = _hi_lo(bh)

    def mid(W, b):
        wp = pad(W, HP, HP, scale=-2.0).astype(_bf16)
        bh = np.zeros(HP, np.float64)
        bh[:H] = np.asarray(b, np.float64) + np.asarray(W, np.float64).sum(axis=0)
        hi, lo = _hi_lo(bh)
        wp[H] = 2.0 * hi.astype(np.float64)
        wp[H + 1] = 2.0 * lo.astype(np.float64)
        return wp.reshape(2, P, HP)

    w2p = mid(W2, b2)
    w3p = mid(W3, b3)
    wrp = pad(Wr, HP, DOUT).astype(_bf16).reshape(2, P, DOUT)

    base = (np.asarray(br, np.float64) + np.asarray(Wr, np.float64).sum(axis=0)).astype(
        np.float32
    )
    eye = np.eye(P, dtype=_bf16)
    return {"w1": w1p_bf, "w2": w2p, "w3": w3p, "wr": wrp, "eye": eye}, base


def _prep_x(x):
    """[B,C,S,M] f32 -> per-core [DA, S*bc] bf16 list (with two ones-rows)."""
    x = np.asarray(x, np.float32)
    B = x.shape[0]
    bc = B // NCORES
    xt = np.ascontiguousarray(x.transpose(1, 3, 2, 0)).reshape(D, S, B).astype(_bf16)
    outs = []
    for i in range(NCORES):
        xc = np.ones((DA, S * bc), dtype=_bf16)
        xc[:D] = xt[:, :, i * bc : (i + 1) * bc].reshape(D, S * bc)
        outs.append(xc)
    return outs


def _postprocess(R_list, base):
    """R per core [12, 2*bc] (pair-interleaved) -> full [B, 12] log_softmax."""
    outs = []
    for R in R_list:
        bc = R.shape[1] // 2
        Rs = (R[:, :bc] + R[:, bc:]).astype(np.float32)
        o = base[None, :] - (2.0 / S) * Rs.T
        m = o.max(axis=1, keepdims=True)
        z = o - m
        lse = np.log(np.exp(z).sum(axis=1, keepdims=True))
        outs.append(z - lse)
    return np.concatenate(outs, axis=0).astype(np.float32)


def _ensure_ntff_hook():
    """Inject antenv.axon_hooks (NTFF profile hook) if the image lacks it."""
    import sys
    try:
        from antenv.axon_hooks import get_axon_ntff_profile_hook  # noqa: F401
        return True
    except ImportError:
        pass
    import contextlib
    import ctypes
    import types

    so_path = "/opt/axon/libaxon_pjrt.so"
    try:
        lib = ctypes.CDLL(so_path)
    except OSError:
        return False
    if not hasattr(lib, "axon_start_nrt_profile"):
        return False
    lib.axon_start_nrt_profile.argtypes = [
        ctypes.POINTER(ctypes.c_int64),
        ctypes.c_size_t,
    ]
    lib.axon_start_nrt_profile.restype = ctypes.c_int64
    lib.axon_stop_nrt_profile.argtypes = [ctypes.c_char_p]
    lib.axon_stop_nrt_profile.restype = ctypes.c_int64

    @contextlib.contextmanager
    def _hook(output_dir, device_ids):
        import jax

        jax.devices()
        if device_ids:
            ids = (ctypes.c_int64 * len(device_ids))(*device_ids)
            rc = lib.axon_start_nrt_profile(ids, len(device_ids))
        else:
            rc = lib.axon_start_nrt_profile(None, 0)
        if rc != 0:
            raise RuntimeError(f"axon_start_nrt_profile rc={rc}")
        try:
            yield
        finally:
            n = lib.axon_stop_nrt_profile(str(output_dir).encode())
            if n < 0:
                raise RuntimeError(f"axon_stop_nrt_profile rc={n}")

    mod = types.ModuleType("antenv.axon_hooks")
    mod._hook = _hook
    mod.get_axon_ntff_profile_hook = lambda: _hook
    mod.set_axon_ntff_profile_hook = lambda h: setattr(mod, "_hook", h)
    import antenv

    sys.modules["antenv.axon_hooks"] = mod
    antenv.axon_hooks = mod
    return True


def kernel(x, W1, b1, W2, b2, W3, b3, Wr, br, _trace=False):
    from concourse.bass_utils import run_bass_kernel_spmd

    if _trace:
        _trace = _ensure_ntff_hook()
    nc = _get_nc()
    wmap, base = _prep_weights(W1, b1, W2, b2, W3, b3, Wr, br)
    xs = _prep_x(x)
    in_maps = [{**wmap, "x": xs[i]} for i in range(NCORES)]
    res = run_bass_kernel_spmd(
        nc, in_maps, core_ids=list(range(NCORES)), trace=_trace
    )
    R_list = [res.results[i]["out"] for i in range(NCORES)]
    out = _postprocess(R_list, base)
    if _trace:
        kernel.last_exec_time_ns = res.exec_time_ns
        kernel.last_results = res
    return out


kernel.last_exec_time_ns = None
kernel.last_results = None
